# revision 33
# baseline (speedup 1.0000x reference)
"""Trainium2 SPMD kernel for LoFTR cross-attention (nn_LoFTRAttention).

Full inputs: x [2,2048,1024], src [2,2048,1024], Wq/Wk/Wv/Wo [1024,1024].
Reference: y = MHA(q=x@Wq.T, k=src@Wk.T, v=src@Wv.T, 16 heads of 64) @ Wo.T

Sharding over 8 NeuronCores: data-parallel on batch (2) x tensor-parallel on
heads (4 groups of 4 heads). Core c: batch c//4, heads [4*(c%4), 4*(c%4)+4).
Each core computes its heads' full attention + its slice of the output
projection (row-split Wo); the host sums the 4 partial outputs per batch
(the all-reduce of the row-split projection).

Device-side layout (per core):
  qT/kT [256, L] (head dims on partitions, pair-major) so the QK^T matmul
  contracts d on the partition dim and lands scores TRANSPOSED ([j, i]) --
  exactly the layout the P@V matmul needs as its moving operand. V is
  augmented with a block of ones columns so each P@V matmul also emits the
  softmax denominator (replicated on partitions 64..127) for free; softmax
  needs no max-subtraction (scores ~ N(0,1); exp stays in fp32 range).
  All matmul operands bf16 (full PE rate), fp32 PSUM accumulate.

Schedule (single region): inputs stream in 512-column chunks (host
pre-arranges each chunk contiguous per partition, 8KB descriptors); the
first attention unit starts after ~2.5MB of DMA. Scores are emitted two
j-steps ahead of P@V so the ACT engine (exp, the per-unit critical
resource) never starves; projection chains and the output projection are
placed as deadline-scheduled fillers in the units' PE slack, and one
independent chain shadows each unit-boundary softmax normalization
(reciprocal_approx_fast + 2 multiplies) so the in-order PE queue doesn't
stall on the single-buffered out PSUM."""

import numpy as np
import ml_dtypes

import concourse.bass as bass
import concourse.mybir as mybir
from concourse.tile import TileContext
from concourse.vector_clock import ScopedClock
from concourse.bass_utils import run_bass_kernel_spmd

F32 = mybir.dt.float32
BF16 = mybir.dt.bfloat16
AF = mybir.ActivationFunctionType
P = 128
ts, ds = bass.ts, bass.ds

B, L, D, NHEAD, DH = 2, 2048, 1024, 16, 64
N_CORES = 8
GROUPS = N_CORES // B          # head groups per batch = 4
HPC = NHEAD // GROUPS          # heads per core = 4
C = HPC * DH                   # per-core projected width = 256
SCORE_MODE = "packed"

_waitsplit_patched = False


def _patch_wait_splitting(maxw=1):
    """This walrus build caps the sem-wait count encodable on a single
    instruction ('Too many sync wait commands'). Split excess waits into
    standalone EventSemaphore instructions (same engine, directly before the
    instruction) at BIR-JSON level, right before the backend compile."""
    global _waitsplit_patched
    if _waitsplit_patched:
        return
    _waitsplit_patched = True
    import orjson
    from concourse import bass2jax, bass_utils

    orig = bass_utils.compile_bir_kernel

    def _split(bir_json):
        d = orjson.loads(bir_json)
        changed = False
        for fn in d.get("functions", []):
            for bb in fn.get("blocks") or []:
                out = []
                for ins in bb.get("instructions", []):
                    si = ins.get("sync_info")
                    waits = (si or {}).get("on_wait") or []
                    if len(waits) > maxw:
                        changed = True
                        for i, w in enumerate(waits[: len(waits) - maxw]):
                            out.append(
                                {
                                    "debug": ins.get("debug", 0),
                                    "engine": ins["engine"],
                                    "ins": [],
                                    "name": f"{ins['name']}-hw{i}",
                                    "opcode": "EventSemaphore",
                                    "outs": [],
                                    "sync_info": {"on_update": [], "on_wait": [w]},
                                }
                            )
                        si["on_wait"] = waits[len(waits) - maxw :]
                    out.append(ins)
                bb["instructions"] = out
        return orjson.dumps(d) if changed else bir_json

    def wrapped(bir_json, tmpdir, neff_name="file.neff"):
        return orig(_split(bir_json), tmpdir, neff_name)

    bass_utils.compile_bir_kernel = wrapped
    bass2jax.compile_bir_kernel = wrapped


_drain_patched = False


def _patch_tile_drain():
    """This walrus build rejects sem waits on the SP Drain instruction
    ('Too many sync wait commands'); emit explicit SP wait_ge's instead."""
    global _drain_patched
    if _drain_patched:
        return
    _drain_patched = True

    def _drain_and_barrier(self, tick_clock, wait_clock):
        nc = self.nc
        nop_inst = nc.sync.nop(nofuse=True)
        wait_clock.add_sem_waits(
            nop_inst.ins, ScopedClock({None: tick_clock.global_clock})
        )
        waits = list(nop_inst.ins.sync_info.on_wait)
        nop_inst.ins.sync_info.on_wait.clear()
        assert self.sems is not None
        num_to_handle = {h.num: h for h in self.sems.allocated().values()}
        for w in waits:
            h = num_to_handle.get(w.id)
            if h is None:
                raise RuntimeError(f"no semaphore handle for drain wait {w}")
            nc.sync.wait_ge(h, w.wait_value)
        nc.sync.drain()
        nc.all_engine_barrier()
        popped = nc._tile_sem_poison_stack.pop()
        assert popped is self._sem_poison
        nc.clear_and_free_semaphores(list(self.sems.allocated().values()))
        nc.all_engine_barrier()

    TileContext._drain_and_barrier = _drain_and_barrier


def build(score_mode="plain"):
    """Per-core Bass program (SPMD: same program, per-core data).

    score_mode: "plain"  - K=64 matmuls in full-array mode
                "packed" - 64-row tiling, two heads concurrent via
                           tile_position (0,0)/(64,0)
    """
    _patch_tile_drain()
    _patch_wait_splitting()
    KO = D // P                 # contraction chunks for projections (8)
    MQ = C // P                 # head pairs (2)
    NJ = L // P                 # key chunks (16)
    NI4 = L // 512              # attention i-units / input chunks (4)
    SW = 512                    # matmul slice width
    CO = C // P                 # final-projection contraction chunks (2)
    NN = D // 512               # output 512-slices (2)
    SCALE = DH ** -0.5
    packed = score_mode == "packed"

    nc = bass.Bass()
    # Inputs pre-chunked on host: [chunk, 128, KO, 512], contiguous per
    # partition line (8KB DMA descriptors).
    xT = nc.declare_dram_parameter("xT", [NI4, P, KO, SW], BF16, isOutput=False)
    srcT = nc.declare_dram_parameter("srcT", [NI4, P, KO, SW], BF16, isOutput=False)
    wqT = nc.declare_dram_parameter("wqT", [P, KO, C], BF16, isOutput=False)
    wkT = nc.declare_dram_parameter("wkT", [P, KO, C], BF16, isOutput=False)
    wvT = nc.declare_dram_parameter("wvT", [P, KO, C], BF16, isOutput=False)
    woT = nc.declare_dram_parameter("woT", [P, CO, D], BF16, isOutput=False)
    y = nc.declare_dram_parameter("y", [L, D], BF16, isOutput=True)

    with TileContext(nc) as tc:
        with (
            tc.tile_pool(name="const", bufs=1) as const_pool,
            tc.tile_pool(name="acts", bufs=1) as acts_pool,
            tc.tile_pool(name="pp_psum", bufs=2, space="PSUM") as pp_psum,
            tc.tile_pool(name="sT_psum", bufs=2, space="PSUM") as sT_psum,
            tc.tile_pool(name="out_psum", bufs=1, space="PSUM") as out_psum,
            tc.tile_pool(name="expS", bufs=4) as expS_pool,
            tc.tile_pool(name="o_sb", bufs=2) as o_pool,
            tc.tile_pool(name="rz", bufs=2) as rz_pool,
            tc.tile_pool(name="y_sb", bufs=4) as y_pool,
        ):
            # chunk-major SBUF layout so each chunk DMA lands contiguous
            xT_sb = const_pool.tile([P, NI4, KO, SW], BF16)
            srcT_sb = const_pool.tile([P, NI4, KO, SW], BF16)
            wq_sb = const_pool.tile([P, KO, C], BF16)
            wk_sb = const_pool.tile([P, KO, C], BF16)
            wv_sb = const_pool.tile([P, KO, C], BF16)
            wo_sb = const_pool.tile([P, CO, D], BF16)
            # DMA issue order = need order: v-proj chunk 0 first, then the
            # q/k chains for unit (0,0), then the rest streaming behind.
            # The sync queue starts transferring ~1.5us before gpsimd's, so
            # the critical prologue set (wv, srcT0, wk, xT0) goes on sync in
            # need order; later chunks stream from the gpsimd queue.
            nc.sync.dma_start(wv_sb[:], wvT[:])
            nc.sync.dma_start(srcT_sb[:, 0], srcT[0])
            nc.sync.dma_start(wk_sb[:], wkT[:])
            nc.sync.dma_start(xT_sb[:, 0], xT[0])
            nc.sync.dma_start(wq_sb[:], wqT[:])
            nc.sync.dma_start(wo_sb[:], woT[:])
            for c in range(1, NI4):
                nc.gpsimd.dma_start(srcT_sb[:, c], srcT[c])
                nc.gpsimd.dma_start(xT_sb[:, c], xT[c])

            qT_sb = acts_pool.tile([P, MQ, L], BF16)
            kT_sb = acts_pool.tile([P, MQ, L], BF16)
            v_sb = acts_pool.tile([P, NJ, HPC, P], BF16)  # [v_h | ones]
            outT_sb = acts_pool.tile([P, MQ, L], BF16)
            nc.vector.memset(v_sb[:, :, :, DH:], 1.0)

            def v_subchunk(c, jlo, jhi):
                for j in range(4 * c + jlo, 4 * c + jhi):
                    pv = pp_psum.tile([P, HPC, DH], F32, tag="pp")
                    pvf = pv.rearrange("p h d -> p (h d)")
                    for k in range(KO):
                        nc.tensor.matmul(
                            pvf,
                            lhsT=srcT_sb[:, c, k, ts(j % 4, P)],
                            rhs=wv_sb[:, k, :],
                            start=(k == 0),
                            stop=(k == KO - 1),
                        )
                    nc.vector.tensor_copy(v_sb[:, j, :, 0:DH], pv[:])

            def v_chunk(c):
                """v = srcT.T @ wvT for key rows [128c*4, 128c*4+512)."""
                v_subchunk(c, 0, 4)

            def _chain_parts(w_sb, act, dst, m, i4):
                """One projection chain as two emission parts (the PSUM
                accumulation group tolerates interleaved unrelated matmuls)."""
                box = {}

                def a():
                    box["t"] = pp_psum.tile(
                        [P, SW], F32, tag="pp", name="ppchain"
                    )
                    for k in range(KO // 2):
                        nc.tensor.matmul(
                            box["t"][:],
                            lhsT=w_sb[:, k, ts(m, P)],
                            rhs=act[:, i4, k, :],
                            start=(k == 0),
                            stop=False,
                        )

                def b():
                    for k in range(KO // 2, KO):
                        nc.tensor.matmul(
                            box["t"][:],
                            lhsT=w_sb[:, k, ts(m, P)],
                            rhs=act[:, i4, k, :],
                            start=False,
                            stop=(k == KO - 1),
                        )
                    nc.vector.tensor_copy(dst[:, m, ts(i4, SW)], box["t"][:])

                return a, b

            def q_parts(m, i4):
                return _chain_parts(wq_sb, xT_sb, qT_sb, m, i4)

            def k_parts(m, c):
                return _chain_parts(wk_sb, srcT_sb, kT_sb, m, c)

            def q_chain(m, i4):
                a, b = q_parts(m, i4)
                a()
                b()

            def k_chain(m, c):
                a, b = k_parts(m, c)
                a()
                b()

            # Output projection split by pair: fp_a computes pair 0's partial
            # (ready one pair-phase early) into SBUF; fp_b adds pair 1's
            # matmul and stores. Moves half the projection into the
            # otherwise-idle units 4-7 and shortens the tail.
            y0_sb = acts_pool.tile([P, L // P, D], BF16)

            def fp_a(i4, lo, hi):
                for ic in range(4 * i4 + lo, 4 * i4 + hi):
                    for n2 in range(NN):
                        py = pp_psum.tile([P, 512], F32, tag="pp")
                        nc.tensor.matmul(
                            py[:],
                            lhsT=outT_sb[:, 0, ts(ic, P)],
                            rhs=wo_sb[:, 0, ts(n2, 512)],
                            start=True,
                            stop=True,
                        )
                        nc.vector.tensor_copy(y0_sb[:, ic, ts(n2, 512)], py[:])

            def fp_b(i4, lo, hi):
                for ic in range(4 * i4 + lo, 4 * i4 + hi):
                    for n2 in range(NN):
                        py = pp_psum.tile([P, 512], F32, tag="pp")
                        nc.tensor.matmul(
                            py[:],
                            lhsT=outT_sb[:, 1, ts(ic, P)],
                            rhs=wo_sb[:, 1, ts(n2, 512)],
                            start=True,
                            stop=True,
                        )
                        ysb = y_pool.tile([P, 512], BF16)
                        nc.vector.tensor_tensor(
                            ysb[:],
                            py[:],
                            y0_sb[:, ic, ts(n2, 512)],
                            mybir.AluOpType.add,
                        )
                        nc.sync.dma_start(y[ts(ic, P), ts(n2, 512)], ysb[:])

            pbs = (0, DH)

            def attn_unit(pair, i4, fillers=None, last=False):
                """One attention unit: both heads of `pair` on a 512-wide
                i-slice. sT/expS tiles are [headA 512 | headB 512].
                Scores+exp are emitted two j-steps ahead of P@V so the ACT
                engine stays saturated. `fillers` maps j -> zero-arg emitter
                run after P@V[j] (deadline: work feeding scores S[j'] must
                sit at slot <= j'-3; work feeding P@V[j'] at slot <= j'-1)."""
                fillers = fillers or {}
                outp = out_psum.tile([P, 2 * SW], F32)  # [A | B], rows 64+: Z

                def scores_exp(j):
                    sT = sT_psum.tile([P, 2 * SW], F32)
                    for s, pb in enumerate(pbs):
                        kw = {"tile_position": (pb, 0)} if packed else {}
                        nc.tensor.matmul(
                            sT[:, ts(s, SW)],
                            lhsT=kT_sb[pb : pb + DH, pair, ts(j, P)],
                            rhs=qT_sb[pb : pb + DH, pair, ts(i4, SW)],
                            start=True,
                            stop=True,
                            **kw,
                        )
                    ex = expS_pool.tile([P, 2 * SW], BF16)
                    # softmax scale pre-folded into Wq on the host
                    nc.scalar.activation(ex[:], sT[:], AF.Exp)
                    return ex

                exs = [scores_exp(0), scores_exp(1)]
                for j in range(NJ):
                    for s in range(2):
                        nc.tensor.matmul(
                            outp[:, ts(s, SW)],
                            lhsT=v_sb[:, j, 2 * pair + s, :],
                            rhs=exs[j][:, ts(s, SW)],
                            start=(j == 0),
                            stop=(j == NJ - 1),
                        )
                    if j + 2 < NJ:
                        exs.append(scores_exp(j + 2))
                    if j in fillers:
                        fillers[j]()
                # Drain PSUM fast (4 DVE copies, shadowed by the next unit's
                # scores + a projection chain), then normalize out/Z later on
                # the otherwise-idle GpSimd engine, off every critical path.
                # Copies land each head's out and Z on the head's home
                # partitions (0:64 / 64:128) so the GpSimd divide sees all
                # its SBUF operands at one base partition.
                osb = o_pool.tile([P, 2, SW], F32)
                if last:
                    # tail: Z copies first on DVE (recip chain head) while
                    # the now-idle ACT engine does the out copies in parallel
                    for s, pb in enumerate(pbs):
                        nc.vector.tensor_copy(
                            osb[pb : pb + DH, 1, :], outp[DH : 2 * DH, ts(s, SW)]
                        )
                    for s, pb in enumerate(pbs):
                        nc.scalar.copy(
                            osb[pb : pb + DH, 0, :], outp[0:DH, ts(s, SW)]
                        )
                else:
                    for s, pb in enumerate(pbs):
                        nc.vector.tensor_copy(
                            osb[pb : pb + DH, 0, :], outp[0:DH, ts(s, SW)]
                        )
                        nc.vector.tensor_copy(
                            osb[pb : pb + DH, 1, :], outp[DH : 2 * DH, ts(s, SW)]
                        )
                rzsb = rz_pool.tile([P, SW], F32)
                nc.vector.reciprocal(rzsb[:], osb[:, 1, :])
                nc.vector.tensor_tensor(
                    outT_sb[:, pair, ts(i4, SW)],
                    osb[:, 0, :],
                    rzsb[:],
                    mybir.AluOpType.mult,
                )

            # ---- emission schedule ----
            # Prologue: just enough projection for unit (0,0) to start.
            v_chunk(0)
            k_chain(0, 0)
            q_chain(0, 0)
            # Forced fillers in unit (0,0): kT pair-0 chunk c feeds S[4c]
            # (deadline slot 4c-3), v chunk c feeds P@V[4c] (slot 4c-1).
            k01, k02, k03 = k_parts(0, 1), k_parts(0, 2), k_parts(0, 3)
            q01 = q_parts(0, 1)
            attn_unit(0, 0, {
                0: k01[0], 1: k01[1],
                2: lambda: v_subchunk(1, 0, 2), 3: lambda: v_subchunk(1, 2, 4),
                4: k02[0], 5: k02[1],
                6: lambda: v_subchunk(2, 0, 2), 7: lambda: v_subchunk(2, 2, 4),
                8: k03[0], 9: k03[1],
                10: lambda: v_subchunk(3, 0, 2), 11: lambda: v_subchunk(3, 2, 4),
                12: q01[0], 13: q01[1],
            })
            q_chain(0, 2)          # shadows unit-boundary normalize
            k10, k11 = k_parts(1, 0), k_parts(1, 1)
            attn_unit(0, 1, {1: k10[0], 5: k10[1], 9: k11[0], 13: k11[1]})
            q_chain(0, 3)
            k12, k13 = k_parts(1, 2), k_parts(1, 3)
            attn_unit(0, 2, {1: k12[0], 5: k12[1], 9: k13[0], 13: k13[1]})
            q_chain(1, 0)
            q11 = q_parts(1, 1)
            attn_unit(0, 3, {1: q11[0], 5: q11[1],
                             8: lambda: fp_a(0, 0, 2),
                             11: lambda: fp_a(0, 2, 4)})
            q_chain(1, 2)
            attn_unit(1, 0, {1: lambda: fp_a(1, 0, 2),
                             4: lambda: fp_a(1, 2, 4)})
            q_chain(1, 3)
            # fp_b(i4) reads the previous unit's normalize output
            # (copies+recip+mult, ~7us latency) -- slots 6+ hide that chain
            attn_unit(1, 1, {1: lambda: fp_a(2, 0, 2),
                             3: lambda: fp_a(2, 2, 4),
                             6: lambda: fp_b(0, 0, 2),
                             10: lambda: fp_b(0, 2, 4)})
            attn_unit(1, 2, {1: lambda: fp_a(3, 0, 2),
                             3: lambda: fp_a(3, 2, 4),
                             6: lambda: fp_b(1, 0, 2),
                             10: lambda: fp_b(1, 2, 4)})
            attn_unit(1, 3, {4: lambda: fp_b(2, 0, 2),
                             8: lambda: fp_b(2, 2, 4)}, last=True)
            fp_b(3, 0, 4)
    return nc


_nc_cache = {}


def get_nc(score_mode=SCORE_MODE):
    if score_mode not in _nc_cache:
        _nc_cache[score_mode] = build(score_mode)
    return _nc_cache[score_mode]


def make_in_maps(x, src, Wq, Wk, Wv, Wo):
    """Host-side sharding: slice weights per head group, transpose + chunk
    activations so every DMA lands contiguous per partition, cast bf16."""
    bf = ml_dtypes.bfloat16
    KO, NI4, SW = D // P, L // 512, 512

    def chunk_act(a):  # [L, D] -> [NI4, 128, KO, 512]; row d = ko*128+p
        aT = np.ascontiguousarray(np.asarray(a, np.float32).T)  # [D, L]
        return np.ascontiguousarray(
            aT.reshape(KO, P, NI4, SW).transpose(2, 1, 0, 3)
        ).astype(bf)

    def chunk_w(wT):  # [D, C] -> [128, KO, C]
        return np.ascontiguousarray(
            wT.reshape(KO, P, -1).transpose(1, 0, 2)
        ).astype(bf)

    x = np.asarray(x, np.float32)
    src = np.asarray(src, np.float32)
    # softmax scale folded into Wq (scores = (x@Wq.T)·(src@Wk.T)/sqrt(dh))
    WqT = np.ascontiguousarray(np.asarray(Wq, np.float32).T) * np.float32(
        DH ** -0.5
    )  # [D, D]
    WkT = np.ascontiguousarray(np.asarray(Wk, np.float32).T)
    WvT = np.ascontiguousarray(np.asarray(Wv, np.float32).T)
    WoTf = np.ascontiguousarray(np.asarray(Wo, np.float32).T)  # [D, D]
    xc = [chunk_act(x[b]) for b in range(B)]
    sc = [chunk_act(src[b]) for b in range(B)]
    in_maps = []
    for c in range(N_CORES):
        b, g = c // GROUPS, c % GROUPS
        cols = slice(C * g, C * (g + 1))
        in_maps.append(
            {
                "xT": xc[b],
                "srcT": sc[b],
                "wqT": chunk_w(WqT[:, cols]),
                "wkT": chunk_w(WkT[:, cols]),
                "wvT": chunk_w(WvT[:, cols]),
                "woT": np.ascontiguousarray(
                    WoTf[cols, :].reshape(C // P, P, D).transpose(1, 0, 2)
                ).astype(bf),
            }
        )
    return in_maps


def assemble(results):
    """Sum the 4 row-split partial projections per batch (host all-reduce)."""
    out = np.zeros((B, L, D), np.float32)
    for c in range(N_CORES):
        out[c // GROUPS] += np.asarray(results[c]["y"], np.float32)
    return out


def kernel(x, src, Wq, Wk, Wv, Wo):
    nc = get_nc()
    in_maps = make_in_maps(x, src, Wq, Wk, Wv, Wo)
    res = run_bass_kernel_spmd(nc, in_maps, list(range(N_CORES)))
    return assemble(res.results)


# revision 42
# speedup vs baseline: 1.0842x; 1.0842x over previous
"""Trainium2 SPMD kernel for LoFTR cross-attention (nn_LoFTRAttention).

Full inputs: x [2,2048,1024], src [2,2048,1024], Wq/Wk/Wv/Wo [1024,1024].
Reference: y = MHA(q=x@Wq.T, k=src@Wk.T, v=src@Wv.T, 16 heads of 64) @ Wo.T

Sharding over 8 NeuronCores: data-parallel on batch (2) x tensor-parallel on
heads (4 groups of 4 heads). Core c: batch c//4, heads [4*(c%4), 4*(c%4)+4).
Each core computes its heads' full attention + its slice of the output
projection (row-split Wo); the host sums the 4 partial outputs per batch
(the all-reduce of the row-split projection).

Device-side layout (per core):
  qT/kT [256, L] (head dims on partitions, pair-major) so the QK^T matmul
  contracts d on the partition dim and lands scores TRANSPOSED ([j, i]) --
  exactly the layout the P@V matmul needs as its moving operand. V is
  augmented with a block of ones columns so each P@V matmul also emits the
  softmax denominator (replicated on partitions 64..127) for free; softmax
  needs no max-subtraction (scores ~ N(0,1); exp stays in fp32 range).
  All matmul operands bf16 (full PE rate), fp32 PSUM accumulate.

Schedule (single region): inputs stream in 512-column chunks (host
pre-arranges each chunk contiguous per partition, 8KB descriptors); the
first attention unit starts after ~2.5MB of DMA. Scores are emitted two
j-steps ahead of P@V so the ACT engine (exp, the per-unit critical
resource) never starves; projection chains and the output projection are
placed as deadline-scheduled fillers in the units' PE slack, and one
independent chain shadows each unit-boundary softmax normalization
(reciprocal_approx_fast + 2 multiplies) so the in-order PE queue doesn't
stall on the single-buffered out PSUM."""

import numpy as np
import ml_dtypes

import concourse.bass as bass
import concourse.mybir as mybir
from concourse.tile import TileContext
from concourse.vector_clock import ScopedClock
from concourse.bass_utils import run_bass_kernel_spmd

F32 = mybir.dt.float32
F16 = mybir.dt.float16
BF16 = mybir.dt.bfloat16
AF = mybir.ActivationFunctionType
P = 128
ts, ds = bass.ts, bass.ds

B, L, D, NHEAD, DH = 2, 2048, 1024, 16, 64
N_CORES = 8
GROUPS = N_CORES // B          # head groups per batch = 4
HPC = NHEAD // GROUPS          # heads per core = 4
C = HPC * DH                   # per-core projected width = 256
SCORE_MODE = "packed"

_waitsplit_patched = False


def _patch_wait_splitting(maxw=1):
    """This walrus build caps the sem-wait count encodable on a single
    instruction ('Too many sync wait commands'). Split excess waits into
    standalone EventSemaphore instructions (same engine, directly before the
    instruction) at BIR-JSON level, right before the backend compile."""
    global _waitsplit_patched
    if _waitsplit_patched:
        return
    _waitsplit_patched = True
    import orjson
    from concourse import bass2jax, bass_utils

    orig = bass_utils.compile_bir_kernel

    def _split(bir_json):
        d = orjson.loads(bir_json)
        changed = False
        for fn in d.get("functions", []):
            for bb in fn.get("blocks") or []:
                out = []
                for ins in bb.get("instructions", []):
                    si = ins.get("sync_info")
                    waits = (si or {}).get("on_wait") or []
                    if len(waits) > maxw:
                        changed = True
                        for i, w in enumerate(waits[: len(waits) - maxw]):
                            out.append(
                                {
                                    "debug": ins.get("debug", 0),
                                    "engine": ins["engine"],
                                    "ins": [],
                                    "name": f"{ins['name']}-hw{i}",
                                    "opcode": "EventSemaphore",
                                    "outs": [],
                                    "sync_info": {"on_update": [], "on_wait": [w]},
                                }
                            )
                        si["on_wait"] = waits[len(waits) - maxw :]
                    out.append(ins)
                bb["instructions"] = out
        return orjson.dumps(d) if changed else bir_json

    def wrapped(bir_json, tmpdir, neff_name="file.neff"):
        return orig(_split(bir_json), tmpdir, neff_name)

    bass_utils.compile_bir_kernel = wrapped
    bass2jax.compile_bir_kernel = wrapped


_drain_patched = False


def _patch_tile_drain():
    """This walrus build rejects sem waits on the SP Drain instruction
    ('Too many sync wait commands'); emit explicit SP wait_ge's instead."""
    global _drain_patched
    if _drain_patched:
        return
    _drain_patched = True

    def _drain_and_barrier(self, tick_clock, wait_clock):
        nc = self.nc
        nop_inst = nc.sync.nop(nofuse=True)
        wait_clock.add_sem_waits(
            nop_inst.ins, ScopedClock({None: tick_clock.global_clock})
        )
        waits = list(nop_inst.ins.sync_info.on_wait)
        nop_inst.ins.sync_info.on_wait.clear()
        assert self.sems is not None
        num_to_handle = {h.num: h for h in self.sems.allocated().values()}
        for w in waits:
            h = num_to_handle.get(w.id)
            if h is None:
                raise RuntimeError(f"no semaphore handle for drain wait {w}")
            nc.sync.wait_ge(h, w.wait_value)
        nc.sync.drain()
        nc.all_engine_barrier()
        popped = nc._tile_sem_poison_stack.pop()
        assert popped is self._sem_poison
        nc.clear_and_free_semaphores(list(self.sems.allocated().values()))
        nc.all_engine_barrier()

    TileContext._drain_and_barrier = _drain_and_barrier


def build(score_mode="plain"):
    """Per-core Bass program (SPMD: same program, per-core data).

    score_mode: "plain"  - K=64 matmuls in full-array mode
                "packed" - 64-row tiling, two heads concurrent via
                           tile_position (0,0)/(64,0)
    """
    _patch_tile_drain()
    _patch_wait_splitting()
    KO = D // P                 # contraction chunks for projections (8)
    MQ = C // P                 # head pairs (2)
    NJ = L // P                 # key chunks (16)
    NI4 = L // 512              # attention i-units / input chunks (4)
    SW = 512                    # matmul slice width
    CO = C // P                 # final-projection contraction chunks (2)
    NN = D // 512               # output 512-slices (2)
    SCALE = DH ** -0.5
    packed = score_mode == "packed"

    nc = bass.Bass()
    # Inputs pre-chunked on host: [chunk, 128, KO, 512], contiguous per
    # partition line (8KB DMA descriptors).
    xT = nc.declare_dram_parameter("xT", [NI4, P, KO, SW], BF16, isOutput=False)
    srcT = nc.declare_dram_parameter("srcT", [NI4, P, KO, SW], BF16, isOutput=False)
    wqT = nc.declare_dram_parameter("wqT", [P, KO, C], BF16, isOutput=False)
    wkT = nc.declare_dram_parameter("wkT", [P, KO, C], BF16, isOutput=False)
    wvT = nc.declare_dram_parameter("wvT", [P, KO, C], BF16, isOutput=False)
    woT = nc.declare_dram_parameter("woT", [P, CO, D], BF16, isOutput=False)
    y = nc.declare_dram_parameter("y", [L, D], BF16, isOutput=True)

    with TileContext(nc) as tc:
        with (
            tc.tile_pool(name="const", bufs=1) as const_pool,
            tc.tile_pool(name="acts", bufs=1) as acts_pool,
            tc.tile_pool(name="pp_psum", bufs=2, space="PSUM") as pp_psum,
            tc.tile_pool(name="sT_psum", bufs=2, space="PSUM") as sT_psum,
            tc.tile_pool(name="out_psum", bufs=1, space="PSUM") as out_psum,
            tc.tile_pool(name="expS", bufs=4) as expS_pool,
            tc.tile_pool(name="o_sb", bufs=2) as o_pool,
            tc.tile_pool(name="rz", bufs=2) as rz_pool,
            tc.tile_pool(name="y_sb", bufs=4) as y_pool,
        ):
            # chunk-major SBUF layout so each chunk DMA lands contiguous
            xT_sb = const_pool.tile([P, NI4, KO, SW], BF16)
            srcT_sb = const_pool.tile([P, NI4, KO, SW], BF16)
            wq_sb = const_pool.tile([P, KO, C], BF16)
            wk_sb = const_pool.tile([P, KO, C], BF16)
            wv_sb = const_pool.tile([P, KO, C], BF16)
            wo_sb = const_pool.tile([P, CO, D], BF16)
            # DMA issue order = need order: v-proj chunk 0 first, then the
            # q/k chains for unit (0,0), then the rest streaming behind.
            # weights issue on the sync queue, activation chunks on the
            # (otherwise idle) gpsimd queue -- parallel issue; interleave so
            # the critical set (wv, wk, srcT0) isn't queued behind the rest
            nc.sync.dma_start(wv_sb[:], wvT[:])
            nc.sync.dma_start(wk_sb[:], wkT[:])
            nc.gpsimd.dma_start(srcT_sb[:, 0], srcT[0])
            nc.sync.dma_start(wq_sb[:], wqT[:])
            nc.gpsimd.dma_start(xT_sb[:, 0], xT[0])
            nc.sync.dma_start(wo_sb[:], woT[:])
            for c in range(1, NI4):
                nc.gpsimd.dma_start(srcT_sb[:, c], srcT[c])
                nc.gpsimd.dma_start(xT_sb[:, c], xT[c])

            qT_sb = acts_pool.tile([P, MQ, L], BF16)
            kT_sb = acts_pool.tile([P, MQ, L], BF16)
            v_sb = acts_pool.tile([P, NJ, HPC, P], BF16)  # [v_h | ones]
            outT_sb = acts_pool.tile([P, MQ, L], BF16)
            nc.vector.memset(v_sb[:, :, :, DH:], 1.0)

            def v_subchunk(c, jlo, jhi):
                for j in range(4 * c + jlo, 4 * c + jhi):
                    pv = pp_psum.tile([P, HPC, DH], F32, tag="pp")
                    pvf = pv.rearrange("p h d -> p (h d)")
                    for k in range(KO):
                        nc.tensor.matmul(
                            pvf,
                            lhsT=srcT_sb[:, c, k, ts(j % 4, P)],
                            rhs=wv_sb[:, k, :],
                            start=(k == 0),
                            stop=(k == KO - 1),
                        )
                    nc.vector.tensor_copy(v_sb[:, j, :, 0:DH], pv[:])

            def v_chunk(c):
                """v = srcT.T @ wvT for key rows [128c*4, 128c*4+512)."""
                v_subchunk(c, 0, 4)

            def _chain_parts(w_sb, act, dst, m, i4):
                """One projection chain as two emission parts (the PSUM
                accumulation group tolerates interleaved unrelated matmuls)."""
                box = {}

                def a():
                    box["t"] = pp_psum.tile(
                        [P, SW], F32, tag="pp", name="ppchain"
                    )
                    for k in range(KO // 2):
                        nc.tensor.matmul(
                            box["t"][:],
                            lhsT=w_sb[:, k, ts(m, P)],
                            rhs=act[:, i4, k, :],
                            start=(k == 0),
                            stop=False,
                        )

                def b():
                    for k in range(KO // 2, KO):
                        nc.tensor.matmul(
                            box["t"][:],
                            lhsT=w_sb[:, k, ts(m, P)],
                            rhs=act[:, i4, k, :],
                            start=False,
                            stop=(k == KO - 1),
                        )
                    nc.vector.tensor_copy(dst[:, m, ts(i4, SW)], box["t"][:])

                return a, b

            def q_parts(m, i4):
                return _chain_parts(wq_sb, xT_sb, qT_sb, m, i4)

            def k_parts(m, c):
                return _chain_parts(wk_sb, srcT_sb, kT_sb, m, c)

            def q_chain(m, i4):
                a, b = q_parts(m, i4)
                a()
                b()

            def k_chain(m, c):
                a, b = k_parts(m, c)
                a()
                b()

            def final_proj(i4, lo=0, hi=SW // P):
                """Output projection for the row-chunks covered by i4."""
                for ic in range(4 * i4 + lo, 4 * i4 + hi):
                    for n2 in range(NN):
                        py = pp_psum.tile([P, 512], F32, tag="pp")
                        for c in range(CO):
                            nc.tensor.matmul(
                                py[:],
                                lhsT=outT_sb[:, c, ts(ic, P)],
                                rhs=wo_sb[:, c, ts(n2, 512)],
                                start=(c == 0),
                                stop=(c == CO - 1),
                            )
                        ysb = y_pool.tile([P, 512], BF16)
                        nc.vector.tensor_copy(ysb[:], py[:])
                        nc.sync.dma_start(y[ts(ic, P), ts(n2, 512)], ysb[:])

            # Tail shortening: pair 0's contribution to the last row block
            # (i4=3) is ready one unit early -- compute it into SBUF there,
            # so after the final unit only pair 1's matmuls + an add remain.
            y0_sb = acts_pool.tile([P, SW // P, D], F32)

            def y0_part(lo=0, hi=4):
                for i, ic in zip(range(lo, hi), range(12 + lo, 12 + hi)):
                    for n2 in range(NN):
                        py = pp_psum.tile([P, 512], F32, tag="pp")
                        nc.tensor.matmul(
                            py[:],
                            lhsT=outT_sb[:, 0, ts(ic, P)],
                            rhs=wo_sb[:, 0, ts(n2, 512)],
                            start=True,
                            stop=True,
                        )
                        nc.vector.tensor_copy(y0_sb[:, i, ts(n2, 512)], py[:])

            def final_tail():
                for i, ic in enumerate(range(12, 16)):
                    for n2 in range(NN):
                        py = pp_psum.tile([P, 512], F32, tag="pp")
                        nc.tensor.matmul(
                            py[:],
                            lhsT=outT_sb[:, 1, ts(ic, P)],
                            rhs=wo_sb[:, 1, ts(n2, 512)],
                            start=True,
                            stop=True,
                        )
                        ysb = y_pool.tile([P, 512], BF16)
                        nc.vector.tensor_tensor(
                            ysb[:],
                            py[:],
                            y0_sb[:, i, ts(n2, 512)],
                            mybir.AluOpType.add,
                        )
                        nc.sync.dma_start(y[ts(ic, P), ts(n2, 512)], ysb[:])

            pbs = (0, DH)

            def attn_unit(pair, i4, fillers=None, last=False):
                """One attention unit: both heads of `pair` on a 512-wide
                i-slice. sT/expS tiles are [headA 512 | headB 512].
                Scores+exp are emitted two j-steps ahead of P@V so the ACT
                engine stays saturated. `fillers` maps j -> zero-arg emitter
                run after P@V[j] (deadline: work feeding scores S[j'] must
                sit at slot <= j'-3; work feeding P@V[j'] at slot <= j'-1)."""
                fillers = fillers or {}
                outp = out_psum.tile([P, 2 * SW], F32)  # [A | B], rows 64+: Z

                def scores_exp(j):
                    sT = sT_psum.tile([P, 2 * SW], F32)
                    for s, pb in enumerate(pbs):
                        kw = {"tile_position": (pb, 0)} if packed else {}
                        nc.tensor.matmul(
                            sT[:, ts(s, SW)],
                            lhsT=kT_sb[pb : pb + DH, pair, ts(j, P)],
                            rhs=qT_sb[pb : pb + DH, pair, ts(i4, SW)],
                            start=True,
                            stop=True,
                            **kw,
                        )
                    ex = expS_pool.tile([P, 2 * SW], BF16)
                    # softmax scale pre-folded into Wq on the host
                    nc.scalar.activation(ex[:], sT[:], AF.Exp)
                    return ex

                exs = [scores_exp(0), scores_exp(1)]
                for j in range(NJ):
                    for s in range(2):
                        nc.tensor.matmul(
                            outp[:, ts(s, SW)],
                            lhsT=v_sb[:, j, 2 * pair + s, :],
                            rhs=exs[j][:, ts(s, SW)],
                            start=(j == 0),
                            stop=(j == NJ - 1),
                        )
                    if j + 2 < NJ:
                        exs.append(scores_exp(j + 2))
                    if j in fillers:
                        fillers[j]()
                # Drain PSUM fast (4 DVE copies, shadowed by the next unit's
                # scores + a projection chain), then normalize out/Z later on
                # the otherwise-idle GpSimd engine, off every critical path.
                # Copies land each head's out and Z on the head's home
                # partitions (0:64 / 64:128) so the GpSimd divide sees all
                # its SBUF operands at one base partition.
                osb = o_pool.tile([P, 2, SW], F32)
                if last:
                    # tail: Z copies first on DVE (recip chain head) while
                    # the now-idle ACT engine does the out copies in parallel
                    for s, pb in enumerate(pbs):
                        nc.vector.tensor_copy(
                            osb[pb : pb + DH, 1, :], outp[DH : 2 * DH, ts(s, SW)]
                        )
                    for s, pb in enumerate(pbs):
                        nc.scalar.copy(
                            osb[pb : pb + DH, 0, :], outp[0:DH, ts(s, SW)]
                        )
                else:
                    for s, pb in enumerate(pbs):
                        nc.vector.tensor_copy(
                            osb[pb : pb + DH, 0, :], outp[0:DH, ts(s, SW)]
                        )
                        nc.vector.tensor_copy(
                            osb[pb : pb + DH, 1, :], outp[DH : 2 * DH, ts(s, SW)]
                        )
                rzsb = rz_pool.tile([P, SW], F32)
                nc.vector.reciprocal(rzsb[:], osb[:, 1, :])
                nc.vector.tensor_tensor(
                    outT_sb[:, pair, ts(i4, SW)],
                    osb[:, 0, :],
                    rzsb[:],
                    mybir.AluOpType.mult,
                )

            # ---- emission schedule ----
            # Prologue: just enough projection for unit (0,0) to start.
            v_chunk(0)
            k_chain(0, 0)
            q_chain(0, 0)
            # Forced fillers in unit (0,0): kT pair-0 chunk c feeds S[4c]
            # (deadline slot 4c-3), v chunk c feeds P@V[4c] (slot 4c-1).
            k01, k02, k03 = k_parts(0, 1), k_parts(0, 2), k_parts(0, 3)
            q01 = q_parts(0, 1)
            attn_unit(0, 0, {
                0: k01[0], 1: k01[1],
                2: lambda: v_subchunk(1, 0, 2), 3: lambda: v_subchunk(1, 2, 4),
                4: k02[0], 5: k02[1],
                6: lambda: v_subchunk(2, 0, 2), 7: lambda: v_subchunk(2, 2, 4),
                8: k03[0], 9: k03[1],
                10: lambda: v_subchunk(3, 0, 2), 11: lambda: v_subchunk(3, 2, 4),
                12: q01[0], 13: q01[1],
            })
            q_chain(0, 2)          # shadows unit-boundary normalize
            k10, k11 = k_parts(1, 0), k_parts(1, 1)
            attn_unit(0, 1, {1: k10[0], 5: k10[1], 9: k11[0], 13: k11[1]})
            q_chain(0, 3)
            k12, k13 = k_parts(1, 2), k_parts(1, 3)
            attn_unit(0, 2, {1: k12[0], 5: k12[1], 9: k13[0], 13: k13[1]})
            q_chain(1, 0)
            q11, q13 = q_parts(1, 1), q_parts(1, 3)
            attn_unit(0, 3, {1: q11[0], 5: q11[1]})
            q_chain(1, 2)
            attn_unit(1, 0, {1: q13[0], 5: q13[1]})
            # final_proj(i4) reads the previous unit's normalize output
            # (copies+recip+mult, ~7us latency) -- slots 4+ hide that chain
            attn_unit(1, 1, {4: lambda: final_proj(0, 0, 2),
                             8: lambda: final_proj(0, 2, 4)})
            attn_unit(1, 2, {4: lambda: final_proj(1, 0, 2),
                             8: lambda: final_proj(1, 2, 4)})
            attn_unit(1, 3, {4: lambda: final_proj(2, 0, 2),
                             8: lambda: final_proj(2, 2, 4),
                             11: lambda: y0_part(0, 2),
                             13: lambda: y0_part(2, 4)}, last=True)
            final_tail()
    return nc


_nc_cache = {}


def get_nc(score_mode=SCORE_MODE):
    if score_mode not in _nc_cache:
        _nc_cache[score_mode] = build(score_mode)
    return _nc_cache[score_mode]


def make_in_maps(x, src, Wq, Wk, Wv, Wo):
    """Host-side sharding: slice weights per head group, transpose + chunk
    activations so every DMA lands contiguous per partition, cast bf16."""
    bf = ml_dtypes.bfloat16
    KO, NI4, SW = D // P, L // 512, 512

    def chunk_act(a):  # [L, D] -> [NI4, 128, KO, 512]; row d = ko*128+p
        aT = np.ascontiguousarray(np.asarray(a, np.float32).T)  # [D, L]
        return np.ascontiguousarray(
            aT.reshape(KO, P, NI4, SW).transpose(2, 1, 0, 3)
        ).astype(bf)

    def chunk_w(wT):  # [D, C] -> [128, KO, C]
        return np.ascontiguousarray(
            wT.reshape(KO, P, -1).transpose(1, 0, 2)
        ).astype(bf)

    x = np.asarray(x, np.float32)
    src = np.asarray(src, np.float32)
    # softmax scale folded into Wq (scores = (x@Wq.T)·(src@Wk.T)/sqrt(dh))
    WqT = np.ascontiguousarray(np.asarray(Wq, np.float32).T) * np.float32(
        DH ** -0.5
    )  # [D, D]
    WkT = np.ascontiguousarray(np.asarray(Wk, np.float32).T)
    WvT = np.ascontiguousarray(np.asarray(Wv, np.float32).T)
    WoTf = np.ascontiguousarray(np.asarray(Wo, np.float32).T)  # [D, D]
    xc = [chunk_act(x[b]) for b in range(B)]
    sc = [chunk_act(src[b]) for b in range(B)]
    in_maps = []
    for c in range(N_CORES):
        b, g = c // GROUPS, c % GROUPS
        cols = slice(C * g, C * (g + 1))
        in_maps.append(
            {
                "xT": xc[b],
                "srcT": sc[b],
                "wqT": chunk_w(WqT[:, cols]),
                "wkT": chunk_w(WkT[:, cols]),
                "wvT": chunk_w(WvT[:, cols]),
                "woT": np.ascontiguousarray(
                    WoTf[cols, :].reshape(C // P, P, D).transpose(1, 0, 2)
                ).astype(bf),
            }
        )
    return in_maps


def assemble(results):
    """Sum the 4 row-split partial projections per batch (host all-reduce)."""
    out = np.zeros((B, L, D), np.float32)
    for c in range(N_CORES):
        out[c // GROUPS] += np.asarray(results[c]["y"], np.float32)
    return out


def kernel(x, src, Wq, Wk, Wv, Wo):
    nc = get_nc()
    in_maps = make_in_maps(x, src, Wq, Wk, Wv, Wo)
    res = run_bass_kernel_spmd(nc, in_maps, list(range(N_CORES)))
    return assemble(res.results)


# revision 47
# speedup vs baseline: 1.0976x; 1.0123x over previous
"""Trainium2 SPMD kernel for LoFTR cross-attention (nn_LoFTRAttention).

Full inputs: x [2,2048,1024], src [2,2048,1024], Wq/Wk/Wv/Wo [1024,1024].
Reference: y = MHA(q=x@Wq.T, k=src@Wk.T, v=src@Wv.T, 16 heads of 64) @ Wo.T

Sharding over 8 NeuronCores: data-parallel on batch (2) x tensor-parallel on
heads (4 groups of 4 heads). Core c: batch c//4, heads [4*(c%4), 4*(c%4)+4).
Each core computes its heads' full attention + its slice of the output
projection (row-split Wo); the host sums the 4 partial outputs per batch
(the all-reduce of the row-split projection).

Device-side layout (per core):
  qT/kT [256, L] (head dims on partitions, pair-major) so the QK^T matmul
  contracts d on the partition dim and lands scores TRANSPOSED ([j, i]) --
  exactly the layout the P@V matmul needs as its moving operand. V is
  augmented with a block of ones columns so each P@V matmul also emits the
  softmax denominator (replicated on partitions 64..127) for free; softmax
  needs no max-subtraction (scores ~ N(0,1); exp stays in fp32 range).
  All matmul operands bf16 (full PE rate), fp32 PSUM accumulate.

Schedule (single region): inputs stream in 512-column chunks (host
pre-arranges each chunk contiguous per partition, 8KB descriptors); the
first attention unit starts after ~2.5MB of DMA. Scores are emitted two
j-steps ahead of P@V so the ACT engine (exp, the per-unit critical
resource) never starves; projection chains and the output projection are
placed as deadline-scheduled fillers in the units' PE slack, and one
independent chain shadows each unit-boundary softmax normalization
(reciprocal_approx_fast + 2 multiplies) so the in-order PE queue doesn't
stall on the single-buffered out PSUM."""

import numpy as np
import ml_dtypes

import concourse.bass as bass
import concourse.mybir as mybir
from concourse.tile import TileContext
from concourse.vector_clock import ScopedClock
from concourse.bass_utils import run_bass_kernel_spmd

F32 = mybir.dt.float32
F16 = mybir.dt.float16
BF16 = mybir.dt.bfloat16
AF = mybir.ActivationFunctionType
P = 128
ts, ds = bass.ts, bass.ds

B, L, D, NHEAD, DH = 2, 2048, 1024, 16, 64
N_CORES = 8
GROUPS = N_CORES // B          # head groups per batch = 4
HPC = NHEAD // GROUPS          # heads per core = 4
C = HPC * DH                   # per-core projected width = 256
SCORE_MODE = "packed"

_waitsplit_patched = False


def _patch_wait_splitting(maxw=1):
    """This walrus build caps the sem-wait count encodable on a single
    instruction ('Too many sync wait commands'). Split excess waits into
    standalone EventSemaphore instructions (same engine, directly before the
    instruction) at BIR-JSON level, right before the backend compile."""
    global _waitsplit_patched
    if _waitsplit_patched:
        return
    _waitsplit_patched = True
    import orjson
    from concourse import bass2jax, bass_utils

    orig = bass_utils.compile_bir_kernel

    def _split(bir_json):
        d = orjson.loads(bir_json)
        changed = False
        for fn in d.get("functions", []):
            for bb in fn.get("blocks") or []:
                out = []
                for ins in bb.get("instructions", []):
                    si = ins.get("sync_info")
                    waits = (si or {}).get("on_wait") or []
                    if len(waits) > maxw:
                        changed = True
                        for i, w in enumerate(waits[: len(waits) - maxw]):
                            out.append(
                                {
                                    "debug": ins.get("debug", 0),
                                    "engine": ins["engine"],
                                    "ins": [],
                                    "name": f"{ins['name']}-hw{i}",
                                    "opcode": "EventSemaphore",
                                    "outs": [],
                                    "sync_info": {"on_update": [], "on_wait": [w]},
                                }
                            )
                        si["on_wait"] = waits[len(waits) - maxw :]
                    out.append(ins)
                bb["instructions"] = out
        return orjson.dumps(d) if changed else bir_json

    def wrapped(bir_json, tmpdir, neff_name="file.neff"):
        return orig(_split(bir_json), tmpdir, neff_name)

    bass_utils.compile_bir_kernel = wrapped
    bass2jax.compile_bir_kernel = wrapped


_drain_patched = False


def _patch_tile_drain():
    """This walrus build rejects sem waits on the SP Drain instruction
    ('Too many sync wait commands'); emit explicit SP wait_ge's instead."""
    global _drain_patched
    if _drain_patched:
        return
    _drain_patched = True

    def _drain_and_barrier(self, tick_clock, wait_clock):
        nc = self.nc
        nop_inst = nc.sync.nop(nofuse=True)
        wait_clock.add_sem_waits(
            nop_inst.ins, ScopedClock({None: tick_clock.global_clock})
        )
        waits = list(nop_inst.ins.sync_info.on_wait)
        nop_inst.ins.sync_info.on_wait.clear()
        assert self.sems is not None
        num_to_handle = {h.num: h for h in self.sems.allocated().values()}
        for w in waits:
            h = num_to_handle.get(w.id)
            if h is None:
                raise RuntimeError(f"no semaphore handle for drain wait {w}")
            nc.sync.wait_ge(h, w.wait_value)
        nc.sync.drain()
        nc.all_engine_barrier()
        popped = nc._tile_sem_poison_stack.pop()
        assert popped is self._sem_poison
        nc.clear_and_free_semaphores(list(self.sems.allocated().values()))
        nc.all_engine_barrier()

    TileContext._drain_and_barrier = _drain_and_barrier


def build(score_mode="plain"):
    """Per-core Bass program (SPMD: same program, per-core data).

    score_mode: "plain"  - K=64 matmuls in full-array mode
                "packed" - 64-row tiling, two heads concurrent via
                           tile_position (0,0)/(64,0)
    """
    _patch_tile_drain()
    _patch_wait_splitting()
    KO = D // P                 # contraction chunks for projections (8)
    MQ = C // P                 # head pairs (2)
    NJ = L // P                 # key chunks (16)
    NI4 = L // 512              # attention i-units / input chunks (4)
    SW = 512                    # matmul slice width
    CO = C // P                 # final-projection contraction chunks (2)
    NN = D // 512               # output 512-slices (2)
    SCALE = DH ** -0.5
    packed = score_mode == "packed"

    nc = bass.Bass()
    # Inputs pre-chunked on host: [chunk, 128, KO, 512], contiguous per
    # partition line (8KB DMA descriptors).
    xT = nc.declare_dram_parameter("xT", [NI4, P, KO, SW], BF16, isOutput=False)
    srcT = nc.declare_dram_parameter("srcT", [NI4, P, KO, SW], BF16, isOutput=False)
    wqT = nc.declare_dram_parameter("wqT", [P, KO, C], BF16, isOutput=False)
    wkT = nc.declare_dram_parameter("wkT", [P, KO, C], BF16, isOutput=False)
    wvT = nc.declare_dram_parameter("wvT", [P, KO, C], BF16, isOutput=False)
    woT = nc.declare_dram_parameter("woT", [P, CO, D], BF16, isOutput=False)
    y = nc.declare_dram_parameter("y", [L, D], BF16, isOutput=True)

    with TileContext(nc) as tc:
        with (
            tc.tile_pool(name="const", bufs=1) as const_pool,
            tc.tile_pool(name="acts", bufs=1) as acts_pool,
            tc.tile_pool(name="pp_psum", bufs=2, space="PSUM") as pp_psum,
            tc.tile_pool(name="sT_psum", bufs=2, space="PSUM") as sT_psum,
            tc.tile_pool(name="out_psum", bufs=1, space="PSUM") as out_psum,
            tc.tile_pool(name="expS", bufs=4) as expS_pool,
            tc.tile_pool(name="o_sb", bufs=2) as o_pool,
            tc.tile_pool(name="rz", bufs=2) as rz_pool,
            tc.tile_pool(name="y_sb", bufs=4) as y_pool,
        ):
            # chunk-major SBUF layout so each chunk DMA lands contiguous
            xT_sb = const_pool.tile([P, NI4, KO, SW], BF16)
            srcT_sb = const_pool.tile([P, NI4, KO, SW], BF16)
            wq_sb = const_pool.tile([P, KO, C], BF16)
            wk_sb = const_pool.tile([P, KO, C], BF16)
            wv_sb = const_pool.tile([P, KO, C], BF16)
            wo_sb = const_pool.tile([P, CO, D], BF16)
            # DMA issue order = need order: v-proj chunk 0 first, then the
            # q/k chains for unit (0,0), then the rest streaming behind.
            # weights issue on the sync queue, activation chunks on the
            # (otherwise idle) gpsimd queue -- parallel issue; interleave so
            # the critical set (wv, wk, srcT0) isn't queued behind the rest
            nc.sync.dma_start(wv_sb[:], wvT[:])
            nc.sync.dma_start(wk_sb[:], wkT[:])
            nc.gpsimd.dma_start(srcT_sb[:, 0], srcT[0])
            nc.sync.dma_start(wq_sb[:], wqT[:])
            nc.gpsimd.dma_start(xT_sb[:, 0], xT[0])
            nc.sync.dma_start(wo_sb[:], woT[:])
            for c in range(1, NI4):
                nc.gpsimd.dma_start(srcT_sb[:, c], srcT[c])
                nc.gpsimd.dma_start(xT_sb[:, c], xT[c])

            qT_sb = acts_pool.tile([P, MQ, L], BF16)
            kT_sb = acts_pool.tile([P, MQ, L], BF16)
            v_sb = acts_pool.tile([P, NJ, HPC, P], BF16)  # [v_h | ones]
            outT_sb = acts_pool.tile([P, MQ, L], BF16)
            nc.vector.memset(v_sb[:, :, :, DH:], 1.0)

            def v_subchunk(c, jlo, jhi):
                for j in range(4 * c + jlo, 4 * c + jhi):
                    pv = pp_psum.tile([P, HPC, DH], F32, tag="pp")
                    pvf = pv.rearrange("p h d -> p (h d)")
                    for k in range(KO):
                        nc.tensor.matmul(
                            pvf,
                            lhsT=srcT_sb[:, c, k, ts(j % 4, P)],
                            rhs=wv_sb[:, k, :],
                            start=(k == 0),
                            stop=(k == KO - 1),
                        )
                    nc.vector.tensor_copy(v_sb[:, j, :, 0:DH], pv[:])

            def v_chunk(c):
                """v = srcT.T @ wvT for key rows [128c*4, 128c*4+512)."""
                v_subchunk(c, 0, 4)

            def _chain_parts(w_sb, act, dst, m, i4):
                """One projection chain as two emission parts (the PSUM
                accumulation group tolerates interleaved unrelated matmuls)."""
                box = {}

                def a():
                    box["t"] = pp_psum.tile(
                        [P, SW], F32, tag="pp", name="ppchain"
                    )
                    for k in range(KO // 2):
                        nc.tensor.matmul(
                            box["t"][:],
                            lhsT=w_sb[:, k, ts(m, P)],
                            rhs=act[:, i4, k, :],
                            start=(k == 0),
                            stop=False,
                        )

                def b():
                    for k in range(KO // 2, KO):
                        nc.tensor.matmul(
                            box["t"][:],
                            lhsT=w_sb[:, k, ts(m, P)],
                            rhs=act[:, i4, k, :],
                            start=False,
                            stop=(k == KO - 1),
                        )
                    nc.vector.tensor_copy(dst[:, m, ts(i4, SW)], box["t"][:])

                return a, b

            def q_parts(m, i4):
                return _chain_parts(wq_sb, xT_sb, qT_sb, m, i4)

            def k_parts(m, c):
                return _chain_parts(wk_sb, srcT_sb, kT_sb, m, c)

            def q_chain(m, i4):
                a, b = q_parts(m, i4)
                a()
                b()

            def k_chain(m, c):
                a, b = k_parts(m, c)
                a()
                b()

            def final_proj(i4, lo=0, hi=SW // P):
                """Output projection for the row-chunks covered by i4."""
                for ic in range(4 * i4 + lo, 4 * i4 + hi):
                    for n2 in range(NN):
                        py = pp_psum.tile([P, 512], F32, tag="pp")
                        for c in range(CO):
                            nc.tensor.matmul(
                                py[:],
                                lhsT=outT_sb[:, c, ts(ic, P)],
                                rhs=wo_sb[:, c, ts(n2, 512)],
                                start=(c == 0),
                                stop=(c == CO - 1),
                            )
                        ysb = y_pool.tile([P, 512], BF16)
                        nc.vector.tensor_copy(ysb[:], py[:])
                        nc.sync.dma_start(y[ts(ic, P), ts(n2, 512)], ysb[:])

            # Tail shortening: pair 0's contribution to the last row block
            # (i4=3) is ready one unit early -- compute it into SBUF there,
            # so after the final unit only pair 1's matmuls + an add remain.
            y0_sb = acts_pool.tile([P, SW // P, D], F32)

            def y0_part(lo=0, hi=4):
                for i, ic in zip(range(lo, hi), range(12 + lo, 12 + hi)):
                    for n2 in range(NN):
                        py = pp_psum.tile([P, 512], F32, tag="pp")
                        nc.tensor.matmul(
                            py[:],
                            lhsT=outT_sb[:, 0, ts(ic, P)],
                            rhs=wo_sb[:, 0, ts(n2, 512)],
                            start=True,
                            stop=True,
                        )
                        nc.vector.tensor_copy(y0_sb[:, i, ts(n2, 512)], py[:])

            def final_tail():
                for i, ic in enumerate(range(12, 16)):
                    for n2 in range(NN):
                        py = pp_psum.tile([P, 512], F32, tag="pp")
                        nc.tensor.matmul(
                            py[:],
                            lhsT=outT_sb[:, 1, ts(ic, P)],
                            rhs=wo_sb[:, 1, ts(n2, 512)],
                            start=True,
                            stop=True,
                        )
                        ysb = y_pool.tile([P, 512], BF16)
                        nc.vector.tensor_tensor(
                            ysb[:],
                            py[:],
                            y0_sb[:, i, ts(n2, 512)],
                            mybir.AluOpType.add,
                        )
                        nc.sync.dma_start(y[ts(ic, P), ts(n2, 512)], ysb[:])

            pbs = (0, DH)

            def attn_unit(pair, i4, fillers=None, last=False, pre=None):
                """One attention unit: both heads of `pair` on a 512-wide
                i-slice. sT/expS tiles are [headA 512 | headB 512].
                Scores+exp are emitted two j-steps ahead of P@V so the ACT
                engine stays saturated. `fillers` maps j -> zero-arg emitter
                run after P@V[j] (deadline: work feeding scores S[j'] must
                sit at slot <= j'-3; work feeding P@V[j'] at slot <= j'-1)."""
                fillers = fillers or {}
                outp = out_psum.tile([P, 2 * SW], F32)  # [A | B], rows 64+: Z

                def scores_exp(j):
                    sT = sT_psum.tile([P, 2 * SW], F32)
                    for s, pb in enumerate(pbs):
                        kw = {"tile_position": (pb, 0)} if packed else {}
                        nc.tensor.matmul(
                            sT[:, ts(s, SW)],
                            lhsT=kT_sb[pb : pb + DH, pair, ts(j, P)],
                            rhs=qT_sb[pb : pb + DH, pair, ts(i4, SW)],
                            start=True,
                            stop=True,
                            **kw,
                        )
                    ex = expS_pool.tile([P, 2 * SW], BF16)
                    # softmax scale pre-folded into Wq on the host
                    nc.scalar.activation(ex[:], sT[:], AF.Exp)
                    return ex

                exs = [scores_exp(0), scores_exp(1)]
                if pre is not None:
                    pre()
                for j in range(NJ):
                    for s in range(2):
                        nc.tensor.matmul(
                            outp[:, ts(s, SW)],
                            lhsT=v_sb[:, j, 2 * pair + s, :],
                            rhs=exs[j][:, ts(s, SW)],
                            start=(j == 0),
                            stop=(j == NJ - 1),
                        )
                    if j + 2 < NJ:
                        exs.append(scores_exp(j + 2))
                    if j in fillers:
                        fillers[j]()
                # Drain PSUM fast (4 DVE copies, shadowed by the next unit's
                # scores + a projection chain), then normalize out/Z later on
                # the otherwise-idle GpSimd engine, off every critical path.
                # Copies land each head's out and Z on the head's home
                # partitions (0:64 / 64:128) so the GpSimd divide sees all
                # its SBUF operands at one base partition.
                osb = o_pool.tile([P, 2, SW], F32)
                if last:
                    # tail: Z copies first on DVE (recip chain head) while
                    # the now-idle ACT engine does the out copies in parallel
                    for s, pb in enumerate(pbs):
                        nc.vector.tensor_copy(
                            osb[pb : pb + DH, 1, :], outp[DH : 2 * DH, ts(s, SW)]
                        )
                    for s, pb in enumerate(pbs):
                        nc.scalar.copy(
                            osb[pb : pb + DH, 0, :], outp[0:DH, ts(s, SW)]
                        )
                else:
                    for s, pb in enumerate(pbs):
                        nc.vector.tensor_copy(
                            osb[pb : pb + DH, 0, :], outp[0:DH, ts(s, SW)]
                        )
                        nc.vector.tensor_copy(
                            osb[pb : pb + DH, 1, :], outp[DH : 2 * DH, ts(s, SW)]
                        )
                rzsb = rz_pool.tile([P, SW], F32)
                nc.vector.reciprocal(rzsb[:], osb[:, 1, :])
                nc.vector.tensor_tensor(
                    outT_sb[:, pair, ts(i4, SW)],
                    osb[:, 0, :],
                    rzsb[:],
                    mybir.AluOpType.mult,
                )

            # ---- emission schedule ----
            # PE warm-up during the input-DMA wait: the tensor engine p-state
            # ramps with continuous busy time, so junk matmuls (on the
            # memset ones block of v_sb) bring the clock up for the prologue.
            for w in range(10):
                pwm = pp_psum.tile([P, HPC, DH], F32, tag="pp", name="warm")
                nc.tensor.matmul(
                    pwm[0:DH, :, :],
                    lhsT=v_sb[:, 0, 0, DH:],
                    rhs=v_sb[:, 0, :, DH:],
                    start=True,
                    stop=True,
                )
            # Prologue: just enough projection for unit (0,0) to start; the
            # v chunk runs inside the unit (after scores 0/1) so the first
            # exp -- the pacing engine -- fires as early as possible.
            k_chain(0, 0)
            q_chain(0, 0)
            # Forced fillers in unit (0,0): kT pair-0 chunk c feeds S[4c]
            # (deadline slot 4c-3), v chunk c feeds P@V[4c] (slot 4c-1).
            k01, k02, k03 = k_parts(0, 1), k_parts(0, 2), k_parts(0, 3)
            q01 = q_parts(0, 1)
            attn_unit(0, 0, {
                0: k01[0], 1: k01[1],
                2: lambda: v_subchunk(1, 0, 2), 3: lambda: v_subchunk(1, 2, 4),
                4: k02[0], 5: k02[1],
                6: lambda: v_subchunk(2, 0, 2), 7: lambda: v_subchunk(2, 2, 4),
                8: k03[0], 9: k03[1],
                10: lambda: v_subchunk(3, 0, 2), 11: lambda: v_subchunk(3, 2, 4),
                12: q01[0], 13: q01[1],
            }, pre=lambda: v_chunk(0))
            q_chain(0, 2)          # shadows unit-boundary normalize
            k10, k11 = k_parts(1, 0), k_parts(1, 1)
            attn_unit(0, 1, {2: k10[0], 7: k10[1], 11: k11[0], 14: k11[1]})
            q_chain(0, 3)
            k12, k13 = k_parts(1, 2), k_parts(1, 3)
            attn_unit(0, 2, {2: k12[0], 7: k12[1], 11: k13[0], 14: k13[1]})
            q_chain(1, 0)
            q11, q13 = q_parts(1, 1), q_parts(1, 3)
            attn_unit(0, 3, {7: q11[0], 14: q11[1]})
            q_chain(1, 2)
            attn_unit(1, 0, {7: q13[0], 14: q13[1]})
            # final_proj(i4) reads the previous unit's normalize output
            # (copies+recip+mult, ~7us latency) -- slots 4+ hide that chain
            attn_unit(1, 1, {4: lambda: final_proj(0, 0, 2),
                             11: lambda: final_proj(0, 2, 4)})
            attn_unit(1, 2, {4: lambda: final_proj(1, 0, 2),
                             11: lambda: final_proj(1, 2, 4)})
            attn_unit(1, 3, {4: lambda: final_proj(2, 0, 2),
                             8: lambda: final_proj(2, 2, 4),
                             11: lambda: y0_part(0, 2),
                             14: lambda: y0_part(2, 4)}, last=True)
            final_tail()
    return nc


_nc_cache = {}


def get_nc(score_mode=SCORE_MODE):
    if score_mode not in _nc_cache:
        _nc_cache[score_mode] = build(score_mode)
    return _nc_cache[score_mode]


def make_in_maps(x, src, Wq, Wk, Wv, Wo):
    """Host-side sharding: slice weights per head group, transpose + chunk
    activations so every DMA lands contiguous per partition, cast bf16."""
    bf = ml_dtypes.bfloat16
    KO, NI4, SW = D // P, L // 512, 512

    def chunk_act(a):  # [L, D] -> [NI4, 128, KO, 512]; row d = ko*128+p
        aT = np.ascontiguousarray(np.asarray(a, np.float32).T)  # [D, L]
        return np.ascontiguousarray(
            aT.reshape(KO, P, NI4, SW).transpose(2, 1, 0, 3)
        ).astype(bf)

    def chunk_w(wT):  # [D, C] -> [128, KO, C]
        return np.ascontiguousarray(
            wT.reshape(KO, P, -1).transpose(1, 0, 2)
        ).astype(bf)

    x = np.asarray(x, np.float32)
    src = np.asarray(src, np.float32)
    # softmax scale folded into Wq (scores = (x@Wq.T)·(src@Wk.T)/sqrt(dh))
    WqT = np.ascontiguousarray(np.asarray(Wq, np.float32).T) * np.float32(
        DH ** -0.5
    )  # [D, D]
    WkT = np.ascontiguousarray(np.asarray(Wk, np.float32).T)
    WvT = np.ascontiguousarray(np.asarray(Wv, np.float32).T)
    WoTf = np.ascontiguousarray(np.asarray(Wo, np.float32).T)  # [D, D]
    xc = [chunk_act(x[b]) for b in range(B)]
    sc = [chunk_act(src[b]) for b in range(B)]
    in_maps = []
    for c in range(N_CORES):
        b, g = c // GROUPS, c % GROUPS
        cols = slice(C * g, C * (g + 1))
        in_maps.append(
            {
                "xT": xc[b],
                "srcT": sc[b],
                "wqT": chunk_w(WqT[:, cols]),
                "wkT": chunk_w(WkT[:, cols]),
                "wvT": chunk_w(WvT[:, cols]),
                "woT": np.ascontiguousarray(
                    WoTf[cols, :].reshape(C // P, P, D).transpose(1, 0, 2)
                ).astype(bf),
            }
        )
    return in_maps


def assemble(results):
    """Sum the 4 row-split partial projections per batch (host all-reduce)."""
    out = np.zeros((B, L, D), np.float32)
    for c in range(N_CORES):
        out[c // GROUPS] += np.asarray(results[c]["y"], np.float32)
    return out


def kernel(x, src, Wq, Wk, Wv, Wo):
    nc = get_nc()
    in_maps = make_in_maps(x, src, Wq, Wk, Wv, Wo)
    res = run_bass_kernel_spmd(nc, in_maps, list(range(N_CORES)))
    return assemble(res.results)
